# revision 22
# baseline (speedup 1.0000x reference)
"""Trainium2 Bass kernel for nn_BasicTransformerBlock (cross-attention block).

Reference computation (per batch b of 16):
  q = x[b] @ Wq                        [4096, 512]
  k/v    = ctx_txt[b] @ Wk/Wv          [77, 512]
  k/v_ip = ctx_img[b] @ Wk_ip/Wv_ip    [16, 512]
  per head h (8 heads, d=64):
    sim = q_h @ k_h.T * 0.125, softmax over keys (txt / img separately)
    out_h = ts * softmax(sim_txt) @ v_txt + is * softmax(sim_img) @ v_img
  out = merge_heads(out) @ Wo + bo     [4096, 320]

Sharding: data-parallel over batch, 2 batches per core on 8 cores.

Kernel structure (per core), all bf16 matmul operands:
  - Keys packed into a 96-wide span: txt at 0:77, dead zeros at 77:80, img
    at 80:96 (32-aligned psum writes).  Dead keys produce exp(0)=1 which is
    corrected by subtracting exactly 3.0 from the text row-sums; their VW
    rows are zero so they are inert in the output matmul.
  - Attention epilogue fused via associativity: VW_h = V_h @ Wo_h [96, 320]
    per head (text/img scales folded into V), so each 128-token output chunk
    is one PSUM accumulation  out = sum_h probsT_h.T @ VW_h.  The bias bo is
    folded into VW head-0 text rows (normalized text probs sum to exactly 1
    per token), so no bias matmul or bias add is needed at all.
  - Streaming pipeline over 16 units (2 batches x 8 groups of 512 tokens):
    load x^T -> Q proj (3 contraction tiles) -> per head-pair: QK^T into a
    2-bank psum -> one Exp per head-pair (scale=0.125, no max-subtraction)
    -> softmax per 4-head HALF (fold-add keys 0:40+40:80 at DVE 2x, text +
    img reduces, -3.0 dead-key fix, bf16 reciprocal, normalize split
    DVE/GPSIMD) -> per-half DMA-xbar transpose (half A issues right after
    exp pair 1, cutting the probs->probsT latency) -> fused out-stage with
    heads 0-3 of both j's before heads 4-7 -> store.
  - The out-stage of unit u is emitted 3 units later (software pipelining)
    so its probsT-waiting matmuls never sit at the PE FIFO head blocking
    ready work: any PE idle gap makes the cost model's p-state drop to
    0.65/1.2 GHz for the next ~3us of decoded matmuls.
  - DMA rings: SP(sync) = x loads + xbar transposes; SWDGE(gpsimd) =
    weights, context, output stores.  Keeps ACT/DVE sequencers free.
  - PSUM: tags qproj(2x1 bank) + sim(2x2 banks) + mm(2x1 bank) = 8 banks.
"""
import sys

if "/opt/trn_rl_repo" not in sys.path:
    sys.path.insert(0, "/opt/trn_rl_repo")

import ml_dtypes
import numpy as np

import concourse.bacc as bacc
import concourse.mybir as mybir
import concourse.tile as tile
from concourse.bass_utils import run_bass_kernel_spmd

F32 = mybir.dt.float32
BF16 = mybir.dt.bfloat16
AF = mybir.ActivationFunctionType
ALU = mybir.AluOpType
X_AX = mybir.AxisListType.X

N_CORES = 8
B = 16
BPC = B // N_CORES          # batches per core
N = 4096                    # tokens
QD = 320                    # query dim
CD = 1024                   # context dim
H = 8                       # heads
D = 64                      # head dim
ID = H * D                  # 512
TXT = 77                    # text keys
IMG = 16                    # image keys
IMG0 = 80                   # key-span offset of img keys
KSPAN = IMG0 + IMG          # 96
NCH = N // 128              # 32 token chunks
NG = NCH // 4               # 8 groups of 4 chunks (512 tokens per unit)
SCALE = 0.125               # 1/sqrt(64)
NRMSPLIT = 44               # keys 0:NRMSPLIT normalized on DVE, rest on Pool

_NC_CACHE = None


def _build_nc():
    nc = bacc.Bacc("TRN2", target_bir_lowering=False, debug=False)

    # x pre-packed on host: x[b, p, c, k, m] = x_orig[b, 128*c+m, 128*k+p]
    x = nc.dram_tensor("x", [BPC, 128, NCH, 3, 128], BF16,
                       kind="ExternalInput").ap()
    # context pre-packed on host: ctx[b, p, k, key] = ctx_orig[b, key', 128*k+p]
    # with txt keys at 0:77, img keys at 80:96, zero padding at 77:80
    ctx = nc.dram_tensor("context", [BPC, 128, 8, KSPAN], BF16,
                         kind="ExternalInput").ap()
    Wq = nc.dram_tensor("Wq", [QD, ID], BF16, kind="ExternalInput").ap()
    Wk = nc.dram_tensor("Wk", [CD, ID], BF16, kind="ExternalInput").ap()
    Wv = nc.dram_tensor("Wv", [CD, ID], BF16, kind="ExternalInput").ap()
    Wk_ip = nc.dram_tensor("Wk_ip", [CD, ID], BF16, kind="ExternalInput").ap()
    Wv_ip = nc.dram_tensor("Wv_ip", [CD, ID], BF16, kind="ExternalInput").ap()
    Wo = nc.dram_tensor("Wo", [ID, QD], BF16, kind="ExternalInput").ap()
    bo = nc.dram_tensor("bo", [QD], BF16, kind="ExternalInput").ap()
    tscale = nc.dram_tensor("text_scale", [1], F32, kind="ExternalInput").ap()
    iscale = nc.dram_tensor("img_scale", [1], F32, kind="ExternalInput").ap()
    out = nc.dram_tensor("out", [BPC, N, QD], F32, kind="ExternalOutput").ap()

    with tile.TileContext(nc) as tc:
        with tc.tile_pool(name="wpool", bufs=1) as wpool, \
             tc.tile_pool(name="kvpool", bufs=2) as kvpool, \
             tc.tile_pool(name="upool", bufs=3) as upool, \
             tc.tile_pool(name="appool", bufs=2) as appool, \
             tc.tile_pool(name="opool", bufs=3) as opool, \
             tc.tile_pool(name="pp", bufs=2, space="PSUM") as pp:

            # ---------------- weights (already bf16 from host) -------------
            def load_w(dram_ap, kt_count, mdim, name):
                wbf = wpool.tile([128, kt_count, mdim], BF16, name=f"w_{name}")
                nc.gpsimd.dma_start(
                    out=wbf[:],
                    in_=dram_ap.rearrange("(k p) m -> p k m", p=128))
                return wbf

            # context tiles first on the SWDGE ring: the K/V projections
            # (and the whole first-unit chain behind them) gate on these
            # small loads, while the big weight tensors stream after.
            ctxts = []
            for b in range(BPC):
                ctxt_b = kvpool.tile([128, 8, KSPAN], BF16, name="ctxt")
                nc.gpsimd.dma_start(out=ctxt_b[:], in_=ctx[b])
                ctxts.append(ctxt_b)

            wq = wpool.tile([128, 3, ID], BF16)
            nc.gpsimd.dma_start(
                out=wq[:, 0:2, :],
                in_=Wq[0:256, :].rearrange("(k p) m -> p k m", p=128))
            nc.gpsimd.dma_start(out=wq[0:64, 2, :], in_=Wq[256:320, :])
            wk = load_w(Wk, 8, ID, "wk")
            wv = load_w(Wv, 8, ID, "wv")
            wkip = load_w(Wk_ip, 8, ID, "wkip")
            wvip = load_w(Wv_ip, 8, ID, "wvip")
            wo = load_w(Wo, 4, QD, "wo")

            bo_row = wpool.tile([1, QD], BF16)
            nc.gpsimd.dma_start(out=bo_row[:], in_=bo[None, :])
            bo_bcast = wpool.tile([128, QD], BF16)
            nc.gpsimd.partition_broadcast(bo_bcast[:], bo_row[:])

            ts_sb = wpool.tile([1, 1], F32)
            nc.gpsimd.dma_start(out=ts_sb[:], in_=tscale[:, None])
            is_sb = wpool.tile([1, 1], F32)
            nc.gpsimd.dma_start(out=is_sb[:], in_=iscale[:, None])
            ts_col = wpool.tile([128, 1], F32)
            nc.gpsimd.partition_broadcast(ts_col[:], ts_sb[:])
            is_col = wpool.tile([128, 1], F32)
            nc.gpsimd.partition_broadcast(is_col[:], is_sb[:])

            kv = []  # per-batch (kt, vw)
            for b in range(BPC):
                # ---------------- context -> K^T, V^T ----------------
                ctxt = ctxts[b]

                kt_ps = pp.tile([128, 512], F32, tag="mm", bufs=2,
                                name="kt_ps")[:, 0:4 * KSPAN].rearrange(
                                    "p (a b) -> p a b", b=KSPAN)
                for m in range(4):
                    for k in range(8):
                        nc.tensor.matmul(
                            kt_ps[:, m, 0:TXT],
                            wk[:, k, 128 * m:128 * (m + 1)],
                            ctxt[:, k, 0:TXT],
                            start=(k == 0), stop=(k == 7))
                for m in range(4):
                    for k in range(8):
                        nc.tensor.matmul(
                            kt_ps[:, m, IMG0:KSPAN],
                            wkip[:, k, 128 * m:128 * (m + 1)],
                            ctxt[:, k, IMG0:KSPAN],
                            start=(k == 0), stop=(k == 7))
                kt = kvpool.tile([128, 4, KSPAN], BF16)
                nc.gpsimd.memset(kt[:], 0.0)
                nc.vector.tensor_copy(kt[:, :, 0:TXT], kt_ps[:, :, 0:TXT])
                nc.vector.tensor_copy(kt[:, :, IMG0:KSPAN],
                                      kt_ps[:, :, IMG0:KSPAN])

                vt_ps = pp.tile([128, 512], F32, tag="mm", bufs=2,
                                name="vt_ps")[:, 0:4 * KSPAN].rearrange(
                                    "p (a b) -> p a b", b=KSPAN)
                for m in range(4):
                    for k in range(8):
                        nc.tensor.matmul(
                            vt_ps[:, m, 0:TXT],
                            wv[:, k, 128 * m:128 * (m + 1)],
                            ctxt[:, k, 0:TXT],
                            start=(k == 0), stop=(k == 7))
                for m in range(4):
                    for k in range(8):
                        nc.tensor.matmul(
                            vt_ps[:, m, IMG0:KSPAN],
                            wvip[:, k, 128 * m:128 * (m + 1)],
                            ctxt[:, k, IMG0:KSPAN],
                            start=(k == 0), stop=(k == 7))
                vt = kvpool.tile([128, 4, KSPAN], BF16)
                nc.gpsimd.memset(vt[:], 0.0)
                nc.vector.tensor_scalar_mul(vt[:, :, 0:TXT],
                                            vt_ps[:, :, 0:TXT],
                                            ts_col[:, 0:1])
                nc.vector.tensor_scalar_mul(vt[:, :, IMG0:KSPAN],
                                            vt_ps[:, :, IMG0:KSPAN],
                                            is_col[:, 0:1])

                # VW_h = V_h @ Wo_h  [96, 320] per head
                vw = kvpool.tile([128, H, QD], BF16)
                for h in range(H):
                    hp, hh = h // 2, h % 2
                    vw_ps = pp.tile([128, 512], F32, tag="mm", bufs=2,
                                    name="vw_ps")
                    nc.tensor.matmul(
                        vw_ps[0:KSPAN, 0:QD],
                        vt[64 * hh:64 * (hh + 1), hp, :],
                        wo[64 * hh:64 * (hh + 1), hp, :],
                        start=True, stop=True)
                    if h % 2 == 0:
                        nc.vector.tensor_copy(vw[0:KSPAN, h, :],
                                              vw_ps[0:KSPAN, 0:QD])
                    else:
                        nc.scalar.activation(vw[0:KSPAN, h, :],
                                             vw_ps[0:KSPAN, 0:QD], AF.Copy)
                # fold bo into head-0 text rows: sum of normalized text
                # probs is exactly 1, so this adds bo to every token's output
                nc.vector.tensor_add(vw[0:TXT, 0, :], vw[0:TXT, 0, :],
                                     bo_bcast[0:TXT, :])
                kv.append((kt, vw))

            # ------------- streaming units: (batch, 512-token group) -------
            # Software-pipelined emission: unit u's out-stage matmuls are
            # emitted inside unit u+1's sim phase so the PE FIFO never has a
            # transpose-waiting out-matmul at its head blocking ready work.
            units = [(b, g) for b in range(BPC) for g in range(NG)]

            def emit_out_pair(st, j0):
                # heads 0-3 (transpose half A) for both j's first, then
                # heads 4-7 (half B), so half-B probsT gets extra slack
                probsT_p, vw_p, out4_p = st["probsT"], st["vw"], st["out4"]
                ps = {}
                for j in (j0, j0 + 1):
                    ps[j] = pp.tile([128, 512], F32, tag="mm", bufs=2,
                                    name="psum_o")
                    for h in range(4):
                        nc.tensor.matmul(
                            ps[j][:, 0:QD],
                            probsT_p[0:KSPAN, 4 * h + j, :],
                            vw_p[0:KSPAN, h, :],
                            start=(h == 0), stop=False)
                for j in (j0, j0 + 1):
                    for h in range(4, H):
                        nc.tensor.matmul(
                            ps[j][:, 0:QD],
                            probsT_p[0:KSPAN, 4 * h + j, :],
                            vw_p[0:KSPAN, h, :],
                            start=False, stop=(h == H - 1))
                for j in (j0, j0 + 1):
                    if j % 2 == 0:
                        nc.scalar.activation(out4_p[:, j, :],
                                             ps[j][:, 0:QD], AF.Copy)
                    else:
                        nc.vector.tensor_copy(out4_p[:, j, :],
                                              ps[j][:, 0:QD])

            def emit_store(st):
                b_p, g_p = st["bg"]
                nc.gpsimd.dma_start(
                    out=out[b_p, 512 * g_p:512 * (g_p + 1), :]
                        .rearrange("(j p) d -> p j d", p=128),
                    in_=st["out4"][:])

            xt_tiles = {}
            xt_tiles[0] = upool.tile([128, 4, 3, 128], BF16, name="xt")
            b0, g0 = units[0]
            nc.sync.dma_start(out=xt_tiles[0][:],
                              in_=x[b0, :, 4 * g0:4 * (g0 + 1), :, :])

            pend = []
            for u, (b, g) in enumerate(units):
                kt, vw = kv[b]
                xt_g = xt_tiles.pop(u)
                if u + 1 < len(units):
                    bn, gn = units[u + 1]
                    xt_tiles[u + 1] = upool.tile([128, 4, 3, 128], BF16,
                                                 name="xt")
                    nc.sync.dma_start(
                        out=xt_tiles[u + 1][:],
                        in_=x[bn, :, 4 * gn:4 * (gn + 1), :, :])

                # Q^T for this unit: [512 (4 m-tiles), 512 tokens]
                qt_g = upool.tile([128, 4, 512], BF16)
                for m in range(4):
                    psum_q = pp.tile([128, 512], F32, tag="qproj", bufs=2)
                    for ki, kp in enumerate((128, 128, 64)):
                        nc.tensor.matmul(
                            psum_q[:],
                            wq[0:kp, ki, 128 * m:128 * (m + 1)],
                            xt_g[0:kp, :, ki, :],
                            start=(ki == 0), stop=(ki == 2))
                    if m % 2 == 0:
                        nc.scalar.activation(qt_g[:, m, :], psum_q[:], AF.Copy)
                    else:
                        nc.vector.tensor_copy(qt_g[:, m, :], psum_q[:])

                # attention scores + exp, one 2-bank psum per head pair;
                # softmax + transpose run per 4-head half (half A issues its
                # xbar transpose right after exp pair 1); previous units'
                # out-stages interleave after pairs 2 and 3
                probs = appool.tile([128, 32, 128], BF16, tag="probs",
                                    bufs=3)
                scr = appool.tile([128, 32, 40], BF16, tag="scr", bufs=2)
                dsum = appool.tile([128, 2, 32], F32, tag="dsum", bufs=2)
                rsum = appool.tile([128, 2, 32], BF16, tag="rsum", bufs=2)
                probsT = appool.tile([128, 32, 128], BF16, tag="probsT",
                                     bufs=4)

                def emit_chain(half):
                    qs = slice(16 * half, 16 * half + 16)
                    nc.vector.tensor_add(scr[:, qs, :], probs[:, qs, 0:40],
                                         probs[:, qs, 40:80])
                    nc.vector.tensor_reduce(out=dsum[:, 0, qs],
                                            in_=scr[:, qs, :],
                                            axis=X_AX, op=ALU.add)
                    nc.vector.tensor_reduce(out=dsum[:, 1, qs],
                                            in_=probs[:, qs, IMG0:KSPAN],
                                            axis=X_AX, op=ALU.add)
                    nc.vector.tensor_scalar_add(dsum[:, 0, qs],
                                                dsum[:, 0, qs], -3.0)
                    with nc.allow_low_precision(reason="bf16 softmax scale"):
                        nc.vector.reciprocal(rsum[:, :, qs], dsum[:, :, qs])
                    # the last units' chains are drain-phase DVE-bound (no
                    # more sim/exp work follows): push their normalize to the
                    # otherwise-idle GPSIMD instead
                    nsp = NRMSPLIT if u < len(units) - 3 else 16
                    nc.vector.tensor_mul(
                        probs[:, qs, 0:nsp], probs[:, qs, 0:nsp],
                        rsum[:, 0, qs][:, :, None]
                            .broadcast_to([128, 16, nsp]))
                    nc.gpsimd.tensor_mul(
                        probs[:, qs, nsp:IMG0],
                        probs[:, qs, nsp:IMG0],
                        rsum[:, 0, qs][:, :, None]
                            .broadcast_to([128, 16, IMG0 - nsp]))
                    img_eng = nc.vector if u < len(units) - 3 else nc.gpsimd
                    img_eng.tensor_mul(
                        probs[:, qs, IMG0:KSPAN], probs[:, qs, IMG0:KSPAN],
                        rsum[:, 1, qs][:, :, None]
                            .broadcast_to([128, 16, IMG]))
                    nc.sync.dma_start(
                        out=probsT[:, qs, :],
                        in_=probs[:, qs, :].rearrange("p q k -> p (q k)"),
                        transpose=True)

                for hp in range(4):
                    psum_s = pp.tile([128, 1024], F32, tag="sim", bufs=2,
                                     name="psum_s").rearrange(
                                         "p (h x) -> p h x", h=2)
                    for hh in range(2):
                        svc = psum_s[:, hh, 0:4 * KSPAN].rearrange(
                            "p (c k) -> p c k", k=KSPAN)
                        for c in range(4):
                            nc.tensor.matmul(
                                svc[:, c, :],
                                qt_g[64 * hh:64 * (hh + 1), hp,
                                     128 * c:128 * (c + 1)],
                                kt[64 * hh:64 * (hh + 1), hp, :],
                                start=True, stop=True)
                    nc.scalar.activation(
                        probs[:, 8 * hp:8 * hp + 8, 0:KSPAN],
                        psum_s[:, :, 0:4 * KSPAN].rearrange(
                            "p h (c k) -> p h c k", k=KSPAN),
                        AF.Exp, scale=SCALE)
                    if hp == 1:
                        emit_chain(0)
                    if len(pend) >= 3 and hp >= 2:
                        emit_out_pair(pend[0], 2 * (hp - 2))
                        if hp == 3:
                            emit_store(pend[0])
                            pend.pop(0)
                emit_chain(1)

                out4 = opool.tile([128, 4, QD], F32, bufs=4)
                pend.append({"probsT": probsT, "vw": vw, "out4": out4,
                             "bg": (b, g)})

            # drain the last units' out-stages
            for st in pend:
                emit_out_pair(st, 0)
                emit_out_pair(st, 2)
                emit_store(st)

    nc.compile()
    return nc


def _get_nc():
    global _NC_CACHE
    if _NC_CACHE is None:
        _NC_CACHE = _build_nc()
    return _NC_CACHE


def _pack_x(x):
    # [B, N, QD] f32 -> [B, 128(p), NCH(c), 3(k), 128(m)] bf16,
    # value at [b, p, c, k, m] = x[b, 128*c+m, 128*k+p]
    xbf = np.asarray(x, np.float32).astype(ml_dtypes.bfloat16)
    xbf = xbf.reshape(B, NCH, 128, QD)                  # b, c, m, qd
    xp = np.zeros((B, NCH, 128, 384), ml_dtypes.bfloat16)
    xp[:, :, :, 0:QD] = xbf
    xp = xp.reshape(B, NCH, 128, 3, 128)                # b, c, m, k, p
    return np.ascontiguousarray(xp.transpose(0, 4, 1, 3, 2))


def _pack_ctx(context):
    # [B, 93, CD] f32 -> [B, 128(p), 8(k), 96(key)] bf16 with txt keys at
    # 0:77, img keys at 80:96, zeros at 77:80
    cbf = np.asarray(context, np.float32).astype(ml_dtypes.bfloat16)
    cbf = cbf.reshape(B, 93, 8, 128).transpose(0, 3, 2, 1)  # b, p, k, key93
    cp = np.zeros((B, 128, 8, KSPAN), ml_dtypes.bfloat16)
    cp[:, :, :, 0:TXT] = cbf[:, :, :, 0:TXT]
    cp[:, :, :, IMG0:KSPAN] = cbf[:, :, :, TXT:93]
    return np.ascontiguousarray(cp)


def kernel(x, context, Wq, Wk, Wv, Wk_ip, Wv_ip, Wo, bo, text_scale, img_scale):
    x = _pack_x(x)
    context = _pack_ctx(context)
    bf = lambda a: np.ascontiguousarray(
        np.asarray(a, np.float32).astype(ml_dtypes.bfloat16))
    shared = {
        "Wq": bf(Wq), "Wk": bf(Wk), "Wv": bf(Wv), "Wk_ip": bf(Wk_ip),
        "Wv_ip": bf(Wv_ip), "Wo": bf(Wo), "bo": bf(bo),
        "text_scale": np.asarray(text_scale, np.float32),
        "img_scale": np.asarray(img_scale, np.float32),
    }
    nc = _get_nc()
    in_maps = []
    for c in range(N_CORES):
        m = dict(shared)
        m["x"] = x[BPC * c:BPC * (c + 1)]
        m["context"] = context[BPC * c:BPC * (c + 1)]
        in_maps.append(m)
    res = run_bass_kernel_spmd(nc, in_maps, core_ids=list(range(N_CORES)))
    return np.concatenate([res.results[c]["out"] for c in range(N_CORES)], axis=0)


# revision 23
# speedup vs baseline: 1.0028x; 1.0028x over previous
"""Trainium2 Bass kernel for nn_BasicTransformerBlock (cross-attention block).

Reference computation (per batch b of 16):
  q = x[b] @ Wq                        [4096, 512]
  k/v    = ctx_txt[b] @ Wk/Wv          [77, 512]
  k/v_ip = ctx_img[b] @ Wk_ip/Wv_ip    [16, 512]
  per head h (8 heads, d=64):
    sim = q_h @ k_h.T * 0.125, softmax over keys (txt / img separately)
    out_h = ts * softmax(sim_txt) @ v_txt + is * softmax(sim_img) @ v_img
  out = merge_heads(out) @ Wo + bo     [4096, 320]

Sharding: data-parallel over batch, 2 batches per core on 8 cores.

Kernel structure (per core), all bf16 matmul operands:
  - Keys packed into a 96-wide span: txt at 0:77, dead zeros at 77:80, img
    at 80:96 (32-aligned psum writes).  Dead keys produce exp(0)=1 which is
    corrected by subtracting exactly 3.0 from the text row-sums; their VW
    rows are zero so they are inert in the output matmul.
  - Attention epilogue fused via associativity: VW_h = V_h @ Wo_h [96, 320]
    per head (text/img scales folded into V), so each 128-token output chunk
    is one PSUM accumulation  out = sum_h probsT_h.T @ VW_h.  The bias bo is
    folded into VW head-0 text rows (normalized text probs sum to exactly 1
    per token), so no bias matmul or bias add is needed at all.
  - Streaming pipeline over 16 units (2 batches x 8 groups of 512 tokens):
    load x^T -> Q proj (3 contraction tiles) -> per head-pair: QK^T into a
    2-bank psum -> one Exp per head-pair (scale=0.125, no max-subtraction)
    -> softmax per 4-head HALF (fold-add keys 0:40+40:80 at DVE 2x, text +
    img reduces, -3.0 dead-key fix, bf16 reciprocal, normalize split
    DVE/GPSIMD) -> per-half DMA-xbar transpose (half A issues right after
    exp pair 1, cutting the probs->probsT latency) -> fused out-stage with
    heads 0-3 of both j's before heads 4-7 -> store.
  - The out-stage of unit u is emitted 3 units later (software pipelining)
    so its probsT-waiting matmuls never sit at the PE FIFO head blocking
    ready work: any PE idle gap makes the cost model's p-state drop to
    0.65/1.2 GHz for the next ~3us of decoded matmuls.
  - DMA rings: SP(sync) = x loads + xbar transposes; SWDGE(gpsimd) =
    weights, context, output stores.  Keeps ACT/DVE sequencers free.
  - PSUM: tags qproj(2x1 bank) + sim(2x2 banks) + mm(2x1 bank) = 8 banks.
"""
import sys

if "/opt/trn_rl_repo" not in sys.path:
    sys.path.insert(0, "/opt/trn_rl_repo")

import ml_dtypes
import numpy as np

import concourse.bacc as bacc
import concourse.mybir as mybir
import concourse.tile as tile
from concourse.bass_utils import run_bass_kernel_spmd

F32 = mybir.dt.float32
BF16 = mybir.dt.bfloat16
AF = mybir.ActivationFunctionType
ALU = mybir.AluOpType
X_AX = mybir.AxisListType.X

N_CORES = 8
B = 16
BPC = B // N_CORES          # batches per core
N = 4096                    # tokens
QD = 320                    # query dim
CD = 1024                   # context dim
H = 8                       # heads
D = 64                      # head dim
ID = H * D                  # 512
TXT = 77                    # text keys
IMG = 16                    # image keys
IMG0 = 80                   # key-span offset of img keys
KSPAN = IMG0 + IMG          # 96
NCH = N // 128              # 32 token chunks
NG = NCH // 4               # 8 groups of 4 chunks (512 tokens per unit)
SCALE = 0.125               # 1/sqrt(64)
NRMSPLIT = 44               # keys 0:NRMSPLIT normalized on DVE, rest on Pool

_NC_CACHE = None


def _build_nc():
    nc = bacc.Bacc("TRN2", target_bir_lowering=False, debug=False)

    # x pre-packed on host: x[b, p, c, k, m] = x_orig[b, 128*c+m, 128*k+p]
    x = nc.dram_tensor("x", [BPC, 128, NCH, 3, 128], BF16,
                       kind="ExternalInput").ap()
    # context pre-packed on host: ctx[b, p, k, key] = ctx_orig[b, key', 128*k+p]
    # with txt keys at 0:77, img keys at 80:96, zero padding at 77:80
    ctx = nc.dram_tensor("context", [BPC, 128, 8, KSPAN], BF16,
                         kind="ExternalInput").ap()
    Wq = nc.dram_tensor("Wq", [QD, ID], BF16, kind="ExternalInput").ap()
    Wk = nc.dram_tensor("Wk", [CD, ID], BF16, kind="ExternalInput").ap()
    Wv = nc.dram_tensor("Wv", [CD, ID], BF16, kind="ExternalInput").ap()
    Wk_ip = nc.dram_tensor("Wk_ip", [CD, ID], BF16, kind="ExternalInput").ap()
    Wv_ip = nc.dram_tensor("Wv_ip", [CD, ID], BF16, kind="ExternalInput").ap()
    Wo = nc.dram_tensor("Wo", [ID, QD], BF16, kind="ExternalInput").ap()
    bo = nc.dram_tensor("bo", [QD], BF16, kind="ExternalInput").ap()
    tscale = nc.dram_tensor("text_scale", [1], F32, kind="ExternalInput").ap()
    iscale = nc.dram_tensor("img_scale", [1], F32, kind="ExternalInput").ap()
    out = nc.dram_tensor("out", [BPC, N, QD], F32, kind="ExternalOutput").ap()

    with tile.TileContext(nc) as tc:
        with tc.tile_pool(name="wpool", bufs=1) as wpool, \
             tc.tile_pool(name="kvpool", bufs=2) as kvpool, \
             tc.tile_pool(name="upool", bufs=3) as upool, \
             tc.tile_pool(name="appool", bufs=2) as appool, \
             tc.tile_pool(name="opool", bufs=3) as opool, \
             tc.tile_pool(name="pp", bufs=2, space="PSUM") as pp:

            # ---------------- weights (already bf16 from host) -------------
            def load_w(dram_ap, kt_count, mdim, name):
                wbf = wpool.tile([128, kt_count, mdim], BF16, name=f"w_{name}")
                nc.gpsimd.dma_start(
                    out=wbf[:],
                    in_=dram_ap.rearrange("(k p) m -> p k m", p=128))
                return wbf

            # context tiles first on the SWDGE ring: the K/V projections
            # (and the whole first-unit chain behind them) gate on these
            # small loads, while the big weight tensors stream after.
            ctxts = []
            for b in range(BPC):
                ctxt_b = kvpool.tile([128, 8, KSPAN], BF16, name="ctxt")
                nc.gpsimd.dma_start(out=ctxt_b[:], in_=ctx[b])
                ctxts.append(ctxt_b)

            wq = wpool.tile([128, 3, ID], BF16)
            nc.gpsimd.dma_start(
                out=wq[:, 0:2, :],
                in_=Wq[0:256, :].rearrange("(k p) m -> p k m", p=128))
            nc.gpsimd.dma_start(out=wq[0:64, 2, :], in_=Wq[256:320, :])
            wk = load_w(Wk, 8, ID, "wk")
            wv = load_w(Wv, 8, ID, "wv")
            wkip = load_w(Wk_ip, 8, ID, "wkip")
            wvip = load_w(Wv_ip, 8, ID, "wvip")
            wo = load_w(Wo, 4, QD, "wo")

            bo_row = wpool.tile([1, QD], BF16)
            nc.gpsimd.dma_start(out=bo_row[:], in_=bo[None, :])
            bo_bcast = wpool.tile([128, QD], BF16)
            nc.gpsimd.partition_broadcast(bo_bcast[:], bo_row[:])

            ts_sb = wpool.tile([1, 1], F32)
            nc.gpsimd.dma_start(out=ts_sb[:], in_=tscale[:, None])
            is_sb = wpool.tile([1, 1], F32)
            nc.gpsimd.dma_start(out=is_sb[:], in_=iscale[:, None])
            ts_col = wpool.tile([128, 1], F32)
            nc.gpsimd.partition_broadcast(ts_col[:], ts_sb[:])
            is_col = wpool.tile([128, 1], F32)
            nc.gpsimd.partition_broadcast(is_col[:], is_sb[:])

            kv = []  # per-batch (kt, vw)
            for b in range(BPC):
                # ---------------- context -> K^T, V^T ----------------
                ctxt = ctxts[b]

                kt_ps = pp.tile([128, 512], F32, tag="mm", bufs=2,
                                name="kt_ps")[:, 0:4 * KSPAN].rearrange(
                                    "p (a b) -> p a b", b=KSPAN)
                for m in range(4):
                    for k in range(8):
                        nc.tensor.matmul(
                            kt_ps[:, m, 0:TXT],
                            wk[:, k, 128 * m:128 * (m + 1)],
                            ctxt[:, k, 0:TXT],
                            start=(k == 0), stop=(k == 7))
                for m in range(4):
                    for k in range(8):
                        nc.tensor.matmul(
                            kt_ps[:, m, IMG0:KSPAN],
                            wkip[:, k, 128 * m:128 * (m + 1)],
                            ctxt[:, k, IMG0:KSPAN],
                            start=(k == 0), stop=(k == 7))
                kt = kvpool.tile([128, 4, KSPAN], BF16)
                nc.gpsimd.memset(kt[:], 0.0)
                nc.vector.tensor_copy(kt[:, :, 0:TXT], kt_ps[:, :, 0:TXT])
                nc.vector.tensor_copy(kt[:, :, IMG0:KSPAN],
                                      kt_ps[:, :, IMG0:KSPAN])

                vt_ps = pp.tile([128, 512], F32, tag="mm", bufs=2,
                                name="vt_ps")[:, 0:4 * KSPAN].rearrange(
                                    "p (a b) -> p a b", b=KSPAN)
                for m in range(4):
                    for k in range(8):
                        nc.tensor.matmul(
                            vt_ps[:, m, 0:TXT],
                            wv[:, k, 128 * m:128 * (m + 1)],
                            ctxt[:, k, 0:TXT],
                            start=(k == 0), stop=(k == 7))
                for m in range(4):
                    for k in range(8):
                        nc.tensor.matmul(
                            vt_ps[:, m, IMG0:KSPAN],
                            wvip[:, k, 128 * m:128 * (m + 1)],
                            ctxt[:, k, IMG0:KSPAN],
                            start=(k == 0), stop=(k == 7))
                vt = kvpool.tile([128, 4, KSPAN], BF16)
                nc.gpsimd.memset(vt[:], 0.0)
                nc.vector.tensor_scalar_mul(vt[:, :, 0:TXT],
                                            vt_ps[:, :, 0:TXT],
                                            ts_col[:, 0:1])
                nc.vector.tensor_scalar_mul(vt[:, :, IMG0:KSPAN],
                                            vt_ps[:, :, IMG0:KSPAN],
                                            is_col[:, 0:1])

                # VW_h = V_h @ Wo_h  [96, 320] per head
                vw = kvpool.tile([128, H, QD], BF16)
                for h in range(H):
                    hp, hh = h // 2, h % 2
                    vw_ps = pp.tile([128, 512], F32, tag="mm", bufs=2,
                                    name="vw_ps")
                    nc.tensor.matmul(
                        vw_ps[0:KSPAN, 0:QD],
                        vt[64 * hh:64 * (hh + 1), hp, :],
                        wo[64 * hh:64 * (hh + 1), hp, :],
                        start=True, stop=True)
                    if h % 2 == 0:
                        nc.vector.tensor_copy(vw[0:KSPAN, h, :],
                                              vw_ps[0:KSPAN, 0:QD])
                    else:
                        nc.scalar.activation(vw[0:KSPAN, h, :],
                                             vw_ps[0:KSPAN, 0:QD], AF.Copy)
                # fold bo into head-0 text rows: sum of normalized text
                # probs is exactly 1, so this adds bo to every token's output
                nc.vector.tensor_add(vw[0:TXT, 0, :], vw[0:TXT, 0, :],
                                     bo_bcast[0:TXT, :])
                kv.append((kt, vw))

            # ------------- streaming units: (batch, 512-token group) -------
            # Software-pipelined emission: unit u's out-stage matmuls are
            # emitted inside unit u+1's sim phase so the PE FIFO never has a
            # transpose-waiting out-matmul at its head blocking ready work.
            units = [(b, g) for b in range(BPC) for g in range(NG)]

            def emit_out_pair(st, j0):
                # heads 0-3 (transpose half A) for both j's first, then
                # heads 4-7 (half B), so half-B probsT gets extra slack
                probsT_p, vw_p, out4_p = st["probsT"], st["vw"], st["out4"]
                ps = {}
                for j in (j0, j0 + 1):
                    ps[j] = pp.tile([128, 512], F32, tag="mm", bufs=2,
                                    name="psum_o")
                    for h in range(4):
                        nc.tensor.matmul(
                            ps[j][:, 0:QD],
                            probsT_p[0:KSPAN, 4 * h + j, :],
                            vw_p[0:KSPAN, h, :],
                            start=(h == 0), stop=False)
                for j in (j0, j0 + 1):
                    for h in range(4, H):
                        nc.tensor.matmul(
                            ps[j][:, 0:QD],
                            probsT_p[0:KSPAN, 4 * h + j, :],
                            vw_p[0:KSPAN, h, :],
                            start=False, stop=(h == H - 1))
                for j in (j0, j0 + 1):
                    if j % 2 == 0:
                        nc.scalar.activation(out4_p[:, j, :],
                                             ps[j][:, 0:QD], AF.Copy)
                    else:
                        nc.vector.tensor_copy(out4_p[:, j, :],
                                              ps[j][:, 0:QD])

            def emit_store(st):
                b_p, g_p = st["bg"]
                nc.gpsimd.dma_start(
                    out=out[b_p, 512 * g_p:512 * (g_p + 1), :]
                        .rearrange("(j p) d -> p j d", p=128),
                    in_=st["out4"][:])

            xt_tiles = {}
            xt_tiles[0] = upool.tile([128, 4, 3, 128], BF16, name="xt")
            b0, g0 = units[0]
            nc.sync.dma_start(out=xt_tiles[0][:],
                              in_=x[b0, :, 4 * g0:4 * (g0 + 1), :, :])

            pend = []
            for u, (b, g) in enumerate(units):
                kt, vw = kv[b]
                xt_g = xt_tiles.pop(u)
                if u + 1 < len(units):
                    bn, gn = units[u + 1]
                    xt_tiles[u + 1] = upool.tile([128, 4, 3, 128], BF16,
                                                 name="xt")
                    nc.sync.dma_start(
                        out=xt_tiles[u + 1][:],
                        in_=x[bn, :, 4 * gn:4 * (gn + 1), :, :])

                # Q^T for this unit: [512 (4 m-tiles), 512 tokens]
                qt_g = upool.tile([128, 4, 512], BF16)
                for m in range(4):
                    psum_q = pp.tile([128, 512], F32, tag="qproj", bufs=2)
                    for ki, kp in enumerate((128, 128, 64)):
                        nc.tensor.matmul(
                            psum_q[:],
                            wq[0:kp, ki, 128 * m:128 * (m + 1)],
                            xt_g[0:kp, :, ki, :],
                            start=(ki == 0), stop=(ki == 2))
                    if m % 2 == 0:
                        nc.scalar.activation(qt_g[:, m, :], psum_q[:], AF.Copy)
                    else:
                        nc.vector.tensor_copy(qt_g[:, m, :], psum_q[:])

                # attention scores + exp, one 2-bank psum per head pair;
                # softmax + transpose run per 4-head half (half A issues its
                # xbar transpose right after exp pair 1); previous units'
                # out-stages interleave after pairs 2 and 3
                probs = appool.tile([128, 32, 128], BF16, tag="probs",
                                    bufs=3)
                scr = appool.tile([128, 32, 40], BF16, tag="scr", bufs=2)
                dsum = appool.tile([128, 2, 32], F32, tag="dsum", bufs=2)
                rsum = appool.tile([128, 2, 32], BF16, tag="rsum", bufs=2)
                probsT = appool.tile([128, 32, 128], BF16, tag="probsT",
                                     bufs=4)

                def emit_chain(half):
                    qs = slice(16 * half, 16 * half + 16)
                    nc.vector.tensor_add(scr[:, qs, :], probs[:, qs, 0:40],
                                         probs[:, qs, 40:80])
                    nc.vector.tensor_reduce(out=dsum[:, 0, qs],
                                            in_=scr[:, qs, :],
                                            axis=X_AX, op=ALU.add)
                    nc.vector.tensor_reduce(out=dsum[:, 1, qs],
                                            in_=probs[:, qs, IMG0:KSPAN],
                                            axis=X_AX, op=ALU.add)
                    nc.vector.tensor_scalar_add(dsum[:, 0, qs],
                                                dsum[:, 0, qs], -3.0)
                    with nc.allow_low_precision(reason="bf16 softmax scale"):
                        nc.vector.reciprocal(rsum[:, :, qs], dsum[:, :, qs])
                    nc.vector.tensor_mul(
                        probs[:, qs, 0:NRMSPLIT], probs[:, qs, 0:NRMSPLIT],
                        rsum[:, 0, qs][:, :, None]
                            .broadcast_to([128, 16, NRMSPLIT]))
                    nc.gpsimd.tensor_mul(
                        probs[:, qs, NRMSPLIT:IMG0],
                        probs[:, qs, NRMSPLIT:IMG0],
                        rsum[:, 0, qs][:, :, None]
                            .broadcast_to([128, 16, IMG0 - NRMSPLIT]))
                    nc.vector.tensor_mul(
                        probs[:, qs, IMG0:KSPAN], probs[:, qs, IMG0:KSPAN],
                        rsum[:, 1, qs][:, :, None]
                            .broadcast_to([128, 16, IMG]))
                    nc.sync.dma_start(
                        out=probsT[:, qs, :],
                        in_=probs[:, qs, :].rearrange("p q k -> p (q k)"),
                        transpose=True)

                for hp in range(4):
                    psum_s = pp.tile([128, 1024], F32, tag="sim", bufs=2,
                                     name="psum_s").rearrange(
                                         "p (h x) -> p h x", h=2)
                    for hh in range(2):
                        svc = psum_s[:, hh, 0:4 * KSPAN].rearrange(
                            "p (c k) -> p c k", k=KSPAN)
                        for c in range(4):
                            nc.tensor.matmul(
                                svc[:, c, :],
                                qt_g[64 * hh:64 * (hh + 1), hp,
                                     128 * c:128 * (c + 1)],
                                kt[64 * hh:64 * (hh + 1), hp, :],
                                start=True, stop=True)
                    nc.scalar.activation(
                        probs[:, 8 * hp:8 * hp + 8, 0:KSPAN],
                        psum_s[:, :, 0:4 * KSPAN].rearrange(
                            "p h (c k) -> p h c k", k=KSPAN),
                        AF.Exp, scale=SCALE)
                    if hp == 1:
                        emit_chain(0)
                    if len(pend) >= 3 and hp >= 2:
                        emit_out_pair(pend[0], 2 * (hp - 2))
                        if hp == 3:
                            emit_store(pend[0])
                            pend.pop(0)
                emit_chain(1)

                out4 = opool.tile([128, 4, QD], F32, bufs=4)
                pend.append({"probsT": probsT, "vw": vw, "out4": out4,
                             "bg": (b, g)})

            # drain the last units' out-stages
            for st in pend:
                emit_out_pair(st, 0)
                emit_out_pair(st, 2)
                emit_store(st)

    nc.compile()
    return nc


def _get_nc():
    global _NC_CACHE
    if _NC_CACHE is None:
        _NC_CACHE = _build_nc()
    return _NC_CACHE


def _pack_x(x):
    # [B, N, QD] f32 -> [B, 128(p), NCH(c), 3(k), 128(m)] bf16,
    # value at [b, p, c, k, m] = x[b, 128*c+m, 128*k+p]
    xbf = np.asarray(x, np.float32).astype(ml_dtypes.bfloat16)
    xbf = xbf.reshape(B, NCH, 128, QD)                  # b, c, m, qd
    xp = np.zeros((B, NCH, 128, 384), ml_dtypes.bfloat16)
    xp[:, :, :, 0:QD] = xbf
    xp = xp.reshape(B, NCH, 128, 3, 128)                # b, c, m, k, p
    return np.ascontiguousarray(xp.transpose(0, 4, 1, 3, 2))


def _pack_ctx(context):
    # [B, 93, CD] f32 -> [B, 128(p), 8(k), 96(key)] bf16 with txt keys at
    # 0:77, img keys at 80:96, zeros at 77:80
    cbf = np.asarray(context, np.float32).astype(ml_dtypes.bfloat16)
    cbf = cbf.reshape(B, 93, 8, 128).transpose(0, 3, 2, 1)  # b, p, k, key93
    cp = np.zeros((B, 128, 8, KSPAN), ml_dtypes.bfloat16)
    cp[:, :, :, 0:TXT] = cbf[:, :, :, 0:TXT]
    cp[:, :, :, IMG0:KSPAN] = cbf[:, :, :, TXT:93]
    return np.ascontiguousarray(cp)


def kernel(x, context, Wq, Wk, Wv, Wk_ip, Wv_ip, Wo, bo, text_scale, img_scale):
    x = _pack_x(x)
    context = _pack_ctx(context)
    bf = lambda a: np.ascontiguousarray(
        np.asarray(a, np.float32).astype(ml_dtypes.bfloat16))
    shared = {
        "Wq": bf(Wq), "Wk": bf(Wk), "Wv": bf(Wv), "Wk_ip": bf(Wk_ip),
        "Wv_ip": bf(Wv_ip), "Wo": bf(Wo), "bo": bf(bo),
        "text_scale": np.asarray(text_scale, np.float32),
        "img_scale": np.asarray(img_scale, np.float32),
    }
    nc = _get_nc()
    in_maps = []
    for c in range(N_CORES):
        m = dict(shared)
        m["x"] = x[BPC * c:BPC * (c + 1)]
        m["context"] = context[BPC * c:BPC * (c + 1)]
        in_maps.append(m)
    res = run_bass_kernel_spmd(nc, in_maps, core_ids=list(range(N_CORES)))
    return np.concatenate([res.results[c]["out"] for c in range(N_CORES)], axis=0)
